# revision 59
# baseline (speedup 1.0000x reference)
"""Trainium2 Bass kernel: BiDAF-style attention (B=32, C=1024, Q=128, d=768).

Data-parallel over batch: 4 batches per NeuronCore x 8 cores, no collectives.

Math (per batch b):
  sim[c,q]  = x_qc[c,q] + x_c[c] + x_q[q],  x_qc = ctx @ (query*wqc)^T
  P[:,c]    = softmax_q(sim[c,:])   -> c2q = P^T-ish matmul with query
  q2c_w     = softmax_c(max_q sim)  -> q2c = q2c_w @ ctx
  g = [ctx, c2q, ctx*c2q, ctx*q2c]

The kernel is DMA-bandwidth bound (~330 GB/s aggregate per core across the
16 DMA engines), so the layout is chosen to minimize HBM traffic:
  - g1 = ctx is never moved through the device at all; the host already
    holds context and splices it into the final output for free.
  - All large tensors cross HBM in fp16 (ctx, ctxT, qwT, qaug in; the
    [g2|g3|g4] output block out) and the host upcasts. fp16 rounding is
    ~3e-4 rel on this data, far inside the 2e-2 gate.
  - ctx arriving in fp16 also feeds the q2c matmul directly (PE runs fp16
    at full rate), eliminating the DVE cast copies a f32r path would need.

Device-side restructuring:
  - simT kept in [q, c] layout (fp16 operands); exp(simT + x_q) fused on
    ScalarE (bias AP). exp(x_c) cancels in the q-softmax; it is re-applied
    only on the tiny [C]-sized q2c path (host ships exp(x_c)).
  - c2q = E^T @ [query | 1 | 0]: the ones column gives the softmax
    denominator for free; normalization is fused into the PSUM evacuation
    (ScalarE scale) and into the g3 multiply (scalar_tensor_tensor on DVE).
  - max_q E per c-block via PE transpose + free-dim reduce_max, interleaved
    with the c2q block loop so the in-order PE never stalls on the DVE.
  - The c axis is loaded in a perfect-shuffle permutation (row 8p+j on
    partition p) so every DMA is one contiguous chunk per partition AND the
    et-transpose output lines up with the packed ctx layout for q2c/stores.
  - q2c's row result is broadcast across partitions with a DRAM-bounce
    broadcast DMA; its g4 multiplies are software-pipelined one batch back.
  - DMA traffic is spread over the three descriptor rings (ctx loads +
    g4 stores -> SP, g2/g3 stores -> ACT, ctxT/qwT/qaug loads + bounce ->
    SWDGE) and excess Tile sync waits are hoisted onto EventSemaphore
    carriers (this walrus embeds at most one wait per instruction).
"""

import os

# The device run goes through jax's axon PJRT backend. If the calling
# process pinned JAX_PLATFORMS (e.g. to "cpu" for a reference run), make
# sure axon is still visible and preferred.
_jp = os.environ.get("JAX_PLATFORMS")
if _jp is not None and "axon" not in _jp.split(","):
    os.environ["JAX_PLATFORMS"] = "axon," + _jp

import numpy as np

B, C, Q, D = 32, 1024, 128, 768
N_CORES = 8
BPC = B // N_CORES          # batches per core
CBLK = C // 128             # 8 c-blocks of 128
DBLK = D // 128             # 6 d-blocks of 128
QAUG = D + 2                # 770 free cols: [c2q | denom | pad]

LAST_RESULT = None  # BassKernelResults of the most recent device run

# This toolchain's walrus embeds at most one sync wait per engine
# instruction; Tile freely attaches several. Hoist extras onto standalone
# EventSemaphore carriers inserted just before the instruction on the same
# engine — sequencers process their stream in order, so the carrier gates
# everything after it.
_MAX_EMBEDDED_WAITS = 1


def _split_waits(nc):
    import concourse.mybir as mybir

    n = 0
    for f in nc.m.functions:
        for blk in f.blocks:
            new_insts = []
            for inst in blk.instructions:
                si = inst.sync_info
                waits = list(si.on_wait) if si is not None else []
                if len(waits) > _MAX_EMBEDDED_WAITS:
                    keep = waits[-_MAX_EMBEDDED_WAITS:]
                    for w in waits[: len(waits) - _MAX_EMBEDDED_WAITS]:
                        ev = mybir.InstEventSemaphore(
                            name=f"{inst.name}-wsplit{n}", ins=[], outs=[]
                        )
                        ev.engine = inst.engine
                        ev.sync_info = mybir.SyncInfo(on_wait=[w], on_update=[])
                        new_insts.append(ev)
                        n += 1
                    inst.sync_info = mybir.SyncInfo(
                        on_wait=keep, on_update=list(si.on_update)
                    )
                new_insts.append(inst)
            blk.instructions = new_insts
    return n


def build_bass(sim=False):
    """Build the per-core Bass/Tile program. Same program on all 8 cores."""
    from contextlib import ExitStack

    import concourse.bass as bass
    import concourse.tile as tile
    from concourse import mybir

    f32 = mybir.dt.float32
    f16 = mybir.dt.float16
    AF = mybir.ActivationFunctionType
    MULT = mybir.AluOpType.mult
    AX = mybir.AxisListType.X

    if sim:
        from concourse import bacc

        nc = bacc.Bacc(None, target_bir_lowering=False, debug=True)
    else:
        nc = bass.Bass()

    ctx_d = nc.declare_dram_parameter("ctx", [BPC, 128, CBLK, D], f16, isOutput=False)
    ctxT_d = nc.declare_dram_parameter("ctxT", [BPC, 128, DBLK, C], f16, isOutput=False)
    qwT_d = nc.declare_dram_parameter("qwT", [BPC, 128, DBLK, Q], f16, isOutput=False)
    qaug_d = nc.declare_dram_parameter("qaug", [BPC, Q, QAUG], f16, isOutput=False)
    xq_d = nc.declare_dram_parameter("xq", [Q, BPC], f32, isOutput=False)
    exc_d = nc.declare_dram_parameter("exc", [128, BPC, CBLK], f32, isOutput=False)
    ident_d = nc.declare_dram_parameter("ident", [128, 128], f16, isOutput=False)
    # device outputs hold only [g2 | g3] and [g4]; g1 = ctx is spliced in on
    # host. Separate tensors keep each per-partition store chunk contiguous
    # (24KB / 12KB), so one big DMA per batch runs at full packet efficiency.
    g23_d = nc.declare_dram_parameter("g23", [BPC, C, 2 * D], f16, isOutput=True)
    g4_d = nc.declare_dram_parameter("g4", [BPC, C, D], f16, isOutput=True)

    with tile.TileContext(nc) as tc, ExitStack() as es:
        singles = es.enter_context(tc.tile_pool(name="singles", bufs=1))
        big = es.enter_context(tc.tile_pool(name="big", bufs=2))
        ctx_pool = es.enter_context(tc.tile_pool(name="ctxp", bufs=4))
        ctxT_pool = es.enter_context(tc.tile_pool(name="ctxTp", bufs=2))
        epool = es.enter_context(tc.tile_pool(name="epool", bufs=2))
        stg_pool = es.enter_context(tc.tile_pool(name="stg", bufs=2))
        small = es.enter_context(tc.tile_pool(name="small", bufs=8))
        bc_pool = es.enter_context(tc.tile_pool(name="bcast", bufs=3))
        dram = es.enter_context(tc.tile_pool(name="dram", bufs=2, space="DRAM"))
        ps_sim = es.enter_context(tc.tile_pool(name="ps_sim", bufs=1, space="PSUM"))
        ps_c2q = es.enter_context(tc.tile_pool(name="ps_c2q", bufs=2, space="PSUM"))
        ps_et = es.enter_context(tc.tile_pool(name="ps_et", bufs=1, space="PSUM"))
        ps_q2c = es.enter_context(tc.tile_pool(name="ps_q2c", bufs=1, space="PSUM"))

        _g4eng = nc.gpsimd if os.environ.get("KBENCH_G4ENG") == "gpsimd" else nc.vector

        _psbc = os.environ.get("KBENCH_PSBC") == "1"

        identity = singles.tile([128, 128], f16)
        nc.sync.dma_start(identity, ident_d[:, :])
        ones_col = singles.tile([128, 1], f32)
        nc.vector.memset(ones_col, 1.0)
        ones_row = singles.tile([1, 128], f16)
        nc.vector.memset(ones_row, 1.0)
        xq_t = singles.tile([Q, BPC], f32)
        nc.sync.dma_start(xq_t, xq_d[:, :])
        exc_t = singles.tile([128, BPC, CBLK], f32)
        nc.sync.dma_start(exc_t, exc_d[:, :, :])

        def issue_loads(b):
            # ring budget: gpsimd = all loads + bounce (~15MB), sync = g2+g3
            # out (12.6MB), scalar = g4 out (6.3MB, issued only when its deps
            # are long resolved so the ACT sequencer never parks).
            ctxT_t = ctxT_pool.tile([128, DBLK, C], f16, tag="ctxT")
            qwT_t = big.tile([128, DBLK, Q], f16, tag="qwT")
            if b == 0 and os.environ.get("KBENCH_B0SYNC") == "1":
                # batch 0 gates the whole pipeline: give its ctxT the empty
                # sync ring so both rings pull input concurrently at startup
                nc.sync.dma_start(ctxT_t, ctxT_d[b])
            else:
                # ctxT first: it gates the next batch's sim matmuls.
                nc.gpsimd.dma_start(ctxT_t, ctxT_d[b])
            nc.gpsimd.dma_start(qwT_t, qwT_d[b])
            qaug_t = big.tile([Q, QAUG], f16, tag="qaug")
            nc.gpsimd.dma_start(qaug_t, qaug_d[b])
            ctx_t = ctx_pool.tile([128, CBLK, D], f16, tag="ctx")
            nc.gpsimd.dma_start(ctx_t, ctx_d[b])
            return ctx_t, ctxT_t, qwT_t, qaug_t

        def q2c_mm(st):
            """PE q2c matmuls for an OLDER batch whose m2 weights resolved
            long ago -- the PE never waits here."""
            p_ctx, p_m2, p_msum, p_g4r = st
            if _psbc:
                q2c_full = ps_q2c.tile([128, QAUG], f32)
                q2c_ps = q2c_full[0:1, :]
            else:
                q2c_full = None
                q2c_ps = ps_q2c.tile([1, QAUG], f32)
            for blk in range(CBLK):
                for lo, hi in ((0, 512), (512, 768)):
                    nc.tensor.matmul(
                        q2c_ps[:, lo:hi],
                        lhsT=p_m2[:, blk : blk + 1],
                        rhs=p_ctx[:, blk, lo:hi],
                        start=(blk == 0),
                        stop=(blk == CBLK - 1),
                    )
            nc.tensor.matmul(
                q2c_ps[:, D : D + 1], lhsT=ones_col, rhs=p_msum, start=True, stop=True
            )
            return q2c_ps, q2c_full

        def q2c_norm(st, q2c_ps, q2c_full):
            p_ctx, p_m2, p_msum, p_g4r = st
            zr_t = small.tile([1, 1], f32, tag="zr")
            nc.vector.reciprocal(zr_t, q2c_ps[:, D : D + 1])
            q2c_sb = small.tile([1, D], f16, tag="q2c")
            nc.scalar.mul(q2c_sb, q2c_ps[:, 0:D], zr_t)
            if _psbc:
                # rank-1 PE broadcast into the same PSUM banks: ones^T @ row.
                # g4's DVE multiplies read the result straight from PSUM --
                # no DRAM bounce, no SBUF staging, no DMA latency.
                for lo, hi in ((0, 512), (512, D)):
                    nc.tensor.matmul(
                        q2c_full[:, lo:hi],
                        lhsT=ones_row,
                        rhs=q2c_sb[:, lo:hi],
                        start=True,
                        stop=True,
                    )
                return p_ctx, q2c_full[:, 0:D], p_g4r
            # broadcast q2c row to 128 partitions via DRAM bounce
            q2c_dram = dram.tile([1, D], f16)
            nc.gpsimd.dma_start(q2c_dram, q2c_sb)
            bcast_t = bc_pool.tile([128, D], f16, tag="bc")
            q2c_ap = q2c_dram[:, :]
            nc.gpsimd.dma_start(
                bcast_t,
                bass.AP(
                    tensor=q2c_ap.tensor, offset=q2c_ap.offset, ap=[[0, 128], [1, D]]
                ),
            )
            return p_ctx, bcast_t, p_g4r

        def q2c_tail(st):
            return q2c_norm(st, *q2c_mm(st))

        def g4_burst(st):
            """Multiplies only; the store is issued later (from ACT, once
            these mults are guaranteed done, so the issue never parks)."""
            pb_ctx, pb_bcast, pb_gr = st
            stg4 = stg_pool.tile([128, CBLK, D], f16, tag="stg4")
            for blk in range(CBLK):
                _g4eng.tensor_mul(stg4[:, blk, :], pb_ctx[:, blk, :], pb_bcast)
            return pb_gr, stg4

        pend_q2c = None    # batch b-1: m2 ready, q2c matmuls pending
        pend_g4 = None     # batch b-2: broadcast ready, g4 multiplies pending
        pend_store = None  # g4 staging tile whose store issue was deferred
        tiles = issue_loads(0)
        for b in range(BPC):
            ctx_t, ctxT_t, qwT_t, qaug_t = tiles
            # prefetch the next batch's inputs NOW, so their descriptors sit
            # ahead of this batch's late-resolving bounce DMAs in ring order
            if b + 1 < BPC:
                tiles = issue_loads(b + 1)
            g23_r = g23_d[b].rearrange("(p j) d -> p j d", j=CBLK)
            g4_r = g4_d[b].rearrange("(p j) d -> p j d", j=CBLK)

            # deferred g4 store: its multiplies finished last iteration, so
            # this ACT-ring issue finds its waits already satisfied
            if pend_store is not None:
                nc.scalar.dma_start(*pend_store)
                pend_store = None

            # ---- simT[q, c] = (query*wqc) @ ctx^T, then E = exp(simT + x_q)
            E_t = epool.tile([Q, C], f16, tag="E")
            for half in range(2):
                sim_ps = ps_sim.tile([Q, 512], f32, tag="sim")
                for k in range(DBLK):
                    nc.tensor.matmul(
                        sim_ps,
                        lhsT=qwT_t[:, k, :],
                        rhs=ctxT_t[:, k, half * 512 : (half + 1) * 512],
                        start=(k == 0),
                        stop=(k == DBLK - 1),
                    )
                nc.scalar.activation(
                    E_t[:, half * 512 : (half + 1) * 512],
                    sim_ps,
                    AF.Exp,
                    bias=xq_t[:, b : b + 1],
                    scale=1.0,
                )

            # ---- q2c stage for batch b-1, slotted right after sim so the
            # PE stream never idles waiting on this batch's max-chain
            pend_g4_new = q2c_tail(pend_q2c) if pend_q2c is not None else None

            # ---- maxE transposes + c2q matmuls + evac
            m_t = small.tile([128, CBLK], f32, tag="m")
            stg = stg_pool.tile([128, CBLK, 2 * D], f16, tag="stg")
            batch_et = os.environ.get("KBENCH_BATCHET") == "1"
            if batch_et:
                # all 8 transposes into one 1-bank PSUM tile, then a single
                # 3D reduce_max: 7 fewer DVE ops and far fewer sync edges,
                # and m2 is ready before the c2q loop even starts
                et_all = ps_et.tile([128, CBLK, 128], f16, tag="et")
                for blk in range(CBLK):
                    nc.tensor.transpose(
                        et_all[:, blk, :], E_t[:, blk * 128 : (blk + 1) * 128], identity
                    )
                nc.vector.reduce_max(m_t, et_all, axis=AX)
            for blk in range(CBLK):
                if not batch_et:
                    et_ps = ps_et.tile([128, 128], f16, tag="et")
                    nc.tensor.transpose(
                        et_ps, E_t[:, blk * 128 : (blk + 1) * 128], identity
                    )
                    nc.vector.reduce_max(m_t[:, blk : blk + 1], et_ps, axis=AX)

                c2q_ps = ps_c2q.tile([128, QAUG], f32)
                for lo, hi in ((0, 512), (512, QAUG)):
                    nc.tensor.matmul(
                        c2q_ps[:, lo:hi],
                        lhsT=E_t[:, blk * 128 : (blk + 1) * 128],
                        rhs=qaug_t[:, lo:hi],
                        start=True,
                        stop=True,
                    )

                rs_t = small.tile([128, 1], f32, tag="rs")
                nc.vector.reciprocal(rs_t, c2q_ps[:, D : D + 1])
                # g2 = c2q (normalized) -- ScalarE evac with fused scale
                nc.scalar.mul(stg[:, blk, 0:D], c2q_ps[:, 0:D], rs_t)

            # ---- q2c weights for THIS batch (consumed next iteration)
            m2_t = small.tile([128, CBLK], f16, tag="m2")
            nc.vector.tensor_mul(m2_t, m_t, exc_t[:, b, :])
            msum_t = small.tile([128, 1], f32, tag="msum")
            nc.vector.reduce_sum(msum_t, m2_t, axis=AX)

            last = b == BPC - 1
            if last and os.environ.get("KBENCH_TAIL") == "1":
                # ---- final batch: drain with maximum store/compute overlap
                st3 = (ctx_t, m2_t, msum_t, g4_r)
                q2c_ps3, q2c_full3 = q2c_mm(st3)
                half = CBLK // 2
                for blk in range(half):
                    nc.vector.tensor_mul(
                        stg[:, blk, D : 2 * D], stg[:, blk, 0:D], ctx_t[:, blk, :]
                    )
                # g4 of b-2 finished its mults last iteration's slot; store
                # it on sync ahead of the first g23 half
                s = g4_burst(pend_g4)
                nc.sync.dma_start(*s)
                nc.sync.dma_start(g23_r[:, 0:half, :], stg[:, 0:half, :])
                pend_g4_last = q2c_norm(st3, q2c_ps3, q2c_full3)
                for blk in range(half, CBLK):
                    nc.vector.tensor_mul(
                        stg[:, blk, D : 2 * D], stg[:, blk, 0:D], ctx_t[:, blk, :]
                    )
                # second g23 half drains on the scalar ring concurrently
                nc.scalar.dma_start(g23_r[:, half:, :], stg[:, half:, :])
                s = g4_burst(pend_g4_new)
                nc.scalar.dma_start(*s)
                s = g4_burst(pend_g4_last)
                nc.sync.dma_start(*s)
                continue

            if last and not _psbc:
                # start the final batch's q2c/broadcast before the g3 burst
                # so its g4 stage overlaps the remaining DVE work
                pend_g4_last = q2c_tail((ctx_t, m2_t, msum_t, g4_r))

            # g3 = ctx * g2, cheap f16 multiplies on DVE
            if os.environ.get("KBENCH_SPLITST") == "1":
                # ship [g2|g3] in two 1.5MB halves on different rings: the
                # first half starts draining 4 blocks earlier, and the store
                # traffic splits across Q10/Q1 instead of bursting on one
                half = CBLK // 2
                for blk in range(half):
                    nc.vector.tensor_mul(
                        stg[:, blk, D : 2 * D], stg[:, blk, 0:D], ctx_t[:, blk, :]
                    )
                nc.scalar.dma_start(g23_r[:, 0:half, :], stg[:, 0:half, :])
                for blk in range(half, CBLK):
                    nc.vector.tensor_mul(
                        stg[:, blk, D : 2 * D], stg[:, blk, 0:D], ctx_t[:, blk, :]
                    )
                nc.sync.dma_start(g23_r[:, half:, :], stg[:, half:, :])
            else:
                # one contiguous 3MB store per batch, parked on idle Sync
                for blk in range(CBLK):
                    nc.vector.tensor_mul(
                        stg[:, blk, D : 2 * D], stg[:, blk, 0:D], ctx_t[:, blk, :]
                    )
                nc.sync.dma_start(g23_r, stg)

            if _psbc:
                # ---- 1-stage g4: batch b-1's broadcast lives in PSUM and
                # must be consumed before the next tail reuses those banks
                if pend_g4_new is not None:
                    pend_store = g4_burst(pend_g4_new)
                if last:
                    t3 = q2c_tail((ctx_t, m2_t, msum_t, g4_r))
                    s = g4_burst(t3)
                    nc.sync.dma_start(*s)
            else:
                # ---- g4 for batch b-2 (broadcast resolved a full stage ago)
                if pend_g4 is not None:
                    pend_store = g4_burst(pend_g4)
                pend_g4 = pend_g4_new
            pend_q2c = (ctx_t, m2_t, msum_t, g4_r)

        if _psbc:
            if pend_store is not None:
                nc.scalar.dma_start(*pend_store)
        elif os.environ.get("KBENCH_TAIL") != "1":
            # drain the pipeline: g4 of batches BPC-2 and BPC-1
            if pend_store is not None:
                nc.scalar.dma_start(*pend_store)
            s1 = g4_burst(pend_g4)
            nc.scalar.dma_start(*s1)
            s2 = g4_burst(pend_g4_last)
            nc.scalar.dma_start(*s2)

    if not sim:
        _split_waits(nc)
    return nc


def prepare_inputs(context, context_mask, query, query_mask, wq, wc, wqc):
    """Host-side prep: fold weights/masks, transpose, shard across 8 cores."""
    ctx = np.ascontiguousarray(np.asarray(context, dtype=np.float32))
    qry = np.ascontiguousarray(np.asarray(query, dtype=np.float32))
    cmask = np.asarray(context_mask)
    qmask = np.asarray(query_mask)
    wq = np.asarray(wq, dtype=np.float32)
    wc = np.asarray(wc, dtype=np.float32)
    wqc = np.asarray(wqc, dtype=np.float32)

    qw = qry * wqc[None, None, :]
    xq = np.einsum("bqd,d->bq", qry, wq).astype(np.float32)
    xc = np.einsum("bcd,d->bc", ctx, wc).astype(np.float32)
    # Mask folding: masked q -> -1e30 bias inside exp; masked c -> exc=0.
    xq_eff = np.where(qmask == 1, xq, np.float32(-1e30)).astype(np.float32)
    with np.errstate(over="ignore"):
        exc = np.exp(
            np.where(cmask == 1, xc, np.float32(-np.inf)), dtype=np.float32
        )

    # c-axis permutation: E-column e <-> context row rho(e) = 8*(e%128) + e//128.
    # Then the et-transpose output (partition p of chunk t <-> e = t*128+p)
    # lands exactly in the packed ctx layout (partition p, chunk j <-> row 8p+j).
    rho = (8 * (np.arange(C) % 128) + np.arange(C) // 128).astype(np.int64)
    # pctx[b, p, j, :] = ctx[b, 8p+j, :]  (contiguous 12KB per partition)
    pctx = np.ascontiguousarray(ctx.reshape(B, 128, CBLK, D).astype(np.float16))
    # pctxT[b, p, k, e] = ctx[b, rho(e), k*128+p]
    ctx_rho = ctx[:, rho, :]                          # [B, C(e-order), D]
    pctxT = np.ascontiguousarray(
        ctx_rho.transpose(0, 2, 1).reshape(B, DBLK, 128, C).transpose(0, 2, 1, 3)
    ).astype(np.float16)
    # pqwT[b, p, k, q] = qw[b, q, k*128+p]
    qwT = np.ascontiguousarray(qw.transpose(0, 2, 1).astype(np.float32))
    pqwT = np.ascontiguousarray(
        qwT.reshape(B, DBLK, 128, Q).transpose(0, 2, 1, 3)
    ).astype(np.float16)
    qaug = np.concatenate(
        [qry, np.ones((B, Q, 1), np.float32), np.zeros((B, Q, 1), np.float32)],
        axis=2,
    ).astype(np.float16)

    in_maps = []
    for i in range(N_CORES):
        sl = slice(i * BPC, (i + 1) * BPC)
        in_maps.append(
            {
                "ctx": pctx[sl],
                "ctxT": pctxT[sl],
                "qwT": pqwT[sl],
                "qaug": np.ascontiguousarray(qaug[sl]),
                "xq": np.ascontiguousarray(xq_eff[sl].T),
                "exc": np.ascontiguousarray(
                    exc[sl].reshape(BPC, 128, CBLK).transpose(1, 0, 2)
                ),
                "ident": np.eye(128, dtype=np.float16),
            }
        )
    return in_maps


def kernel(context, context_mask, query, query_mask, wq, wc, wqc):
    global LAST_RESULT
    from concourse.bass_utils import run_bass_kernel_spmd

    in_maps = prepare_inputs(
        context, context_mask, query, query_mask, wq, wc, wqc
    )
    nc = build_bass()
    res = run_bass_kernel_spmd(nc, in_maps, core_ids=list(range(N_CORES)))
    LAST_RESULT = res
    out = np.empty((B, C, 4 * D), dtype=np.float32)
    # g1 = context verbatim (never moved through the device)
    out[:, :, 0:D] = np.asarray(context, dtype=np.float32)
    for i in range(N_CORES):
        sl = slice(i * BPC, (i + 1) * BPC)
        out[sl, :, D : 3 * D] = (
            res.results[i]["g23"].reshape(BPC, C, 2 * D).astype(np.float32)
        )
        out[sl, :, 3 * D :] = (
            res.results[i]["g4"].reshape(BPC, C, D).astype(np.float32)
        )
    return out



# revision 62
# speedup vs baseline: 1.1871x; 1.1871x over previous
"""Trainium2 Bass kernel: BiDAF-style attention (B=32, C=1024, Q=128, d=768).

Data-parallel over batch: 4 batches per NeuronCore x 8 cores, no collectives.

Math (per batch b):
  sim[c,q]  = x_qc[c,q] + x_c[c] + x_q[q],  x_qc = ctx @ (query*wqc)^T
  P[:,c]    = softmax_q(sim[c,:])   -> c2q = P^T-ish matmul with query
  q2c_w     = softmax_c(max_q sim)  -> q2c = q2c_w @ ctx
  g = [ctx, c2q, ctx*c2q, ctx*q2c]

The kernel is DMA-bandwidth bound (~330 GB/s aggregate per core across the
16 DMA engines), so the layout is chosen to minimize HBM traffic:
  - g1 = ctx is never moved through the device at all; the host already
    holds context and splices it into the final output for free.
  - All large tensors cross HBM in fp16 (ctx, ctxT, qwT, qaug in; the
    [g2|g3|g4] output block out) and the host upcasts. fp16 rounding is
    ~3e-4 rel on this data, far inside the 2e-2 gate.
  - ctx arriving in fp16 also feeds the q2c matmul directly (PE runs fp16
    at full rate), eliminating the DVE cast copies a f32r path would need.

Device-side restructuring:
  - simT kept in [q, c] layout (fp16 operands); exp(simT + x_q) fused on
    ScalarE (bias AP). exp(x_c) cancels in the q-softmax; it is re-applied
    only on the tiny [C]-sized q2c path (host ships exp(x_c)).
  - c2q = E^T @ [query | 1 | 0]: the ones column gives the softmax
    denominator for free; normalization is fused into the PSUM evacuation
    (ScalarE scale) and into the g3 multiply (scalar_tensor_tensor on DVE).
  - max_q E per c-block via PE transpose + free-dim reduce_max, interleaved
    with the c2q block loop so the in-order PE never stalls on the DVE.
  - The c axis is loaded in a perfect-shuffle permutation (row 8p+j on
    partition p) so every DMA is one contiguous chunk per partition AND the
    et-transpose output lines up with the packed ctx layout for q2c/stores.
  - q2c's row result is broadcast across partitions with a DRAM-bounce
    broadcast DMA; its g4 multiplies are software-pipelined one batch back.
  - DMA traffic is spread over the three descriptor rings (ctx loads +
    g4 stores -> SP, g2/g3 stores -> ACT, ctxT/qwT/qaug loads + bounce ->
    SWDGE) and excess Tile sync waits are hoisted onto EventSemaphore
    carriers (this walrus embeds at most one wait per instruction).
"""

import os

# The device run goes through jax's axon PJRT backend. If the calling
# process pinned JAX_PLATFORMS (e.g. to "cpu" for a reference run), make
# sure axon is still visible and preferred.
_jp = os.environ.get("JAX_PLATFORMS")
if _jp is not None and "axon" not in _jp.split(","):
    os.environ["JAX_PLATFORMS"] = "axon," + _jp

import numpy as np

B, C, Q, D = 32, 1024, 128, 768
N_CORES = 8
BPC = B // N_CORES          # batches per core
CBLK = C // 128             # 8 c-blocks of 128
DBLK = D // 128             # 6 d-blocks of 128
QAUG = D + 2                # 770 free cols: [c2q | denom | pad]

LAST_RESULT = None  # BassKernelResults of the most recent device run

# This toolchain's walrus embeds at most one sync wait per engine
# instruction; Tile freely attaches several. Hoist extras onto standalone
# EventSemaphore carriers inserted just before the instruction on the same
# engine — sequencers process their stream in order, so the carrier gates
# everything after it.
_MAX_EMBEDDED_WAITS = 1


def _split_waits(nc):
    import concourse.mybir as mybir

    n = 0
    for f in nc.m.functions:
        for blk in f.blocks:
            new_insts = []
            for inst in blk.instructions:
                si = inst.sync_info
                waits = list(si.on_wait) if si is not None else []
                if len(waits) > _MAX_EMBEDDED_WAITS:
                    keep = waits[-_MAX_EMBEDDED_WAITS:]
                    for w in waits[: len(waits) - _MAX_EMBEDDED_WAITS]:
                        ev = mybir.InstEventSemaphore(
                            name=f"{inst.name}-wsplit{n}", ins=[], outs=[]
                        )
                        ev.engine = inst.engine
                        ev.sync_info = mybir.SyncInfo(on_wait=[w], on_update=[])
                        new_insts.append(ev)
                        n += 1
                    inst.sync_info = mybir.SyncInfo(
                        on_wait=keep, on_update=list(si.on_update)
                    )
                new_insts.append(inst)
            blk.instructions = new_insts
    return n


def build_bass(sim=False):
    """Build the per-core Bass/Tile program. Same program on all 8 cores."""
    from contextlib import ExitStack

    import concourse.bass as bass
    import concourse.tile as tile
    from concourse import mybir

    f32 = mybir.dt.float32
    f16 = mybir.dt.float16
    AF = mybir.ActivationFunctionType
    MULT = mybir.AluOpType.mult
    AX = mybir.AxisListType.X

    if sim:
        from concourse import bacc

        nc = bacc.Bacc(None, target_bir_lowering=False, debug=True)
    else:
        nc = bass.Bass()

    ctx_d = nc.declare_dram_parameter("ctx", [BPC, 128, CBLK, D], f16, isOutput=False)
    ctxT_d = nc.declare_dram_parameter("ctxT", [BPC, 128, DBLK, C], f16, isOutput=False)
    qwT_d = nc.declare_dram_parameter("qwT", [BPC, 128, DBLK, Q], f16, isOutput=False)
    qaug_d = nc.declare_dram_parameter("qaug", [BPC, Q, QAUG], f16, isOutput=False)
    xq_d = nc.declare_dram_parameter("xq", [Q, BPC], f32, isOutput=False)
    exc_d = nc.declare_dram_parameter("exc", [128, BPC, CBLK], f32, isOutput=False)
    ident_d = nc.declare_dram_parameter("ident", [128, 128], f16, isOutput=False)
    # device outputs hold only [g2 | g3] and [g4]; g1 = ctx is spliced in on
    # host. Separate tensors keep each per-partition store chunk contiguous
    # (24KB / 12KB), so one big DMA per batch runs at full packet efficiency.
    g23_d = nc.declare_dram_parameter("g23", [BPC, C, 2 * D], f16, isOutput=True)
    g4_d = nc.declare_dram_parameter("g4", [BPC, C, D], f16, isOutput=True)

    with tile.TileContext(nc) as tc, ExitStack() as es:
        _pf2 = os.environ.get("KBENCH_PF2") == "1"
        singles = es.enter_context(tc.tile_pool(name="singles", bufs=1))
        big = es.enter_context(tc.tile_pool(name="big", bufs=3 if _pf2 else 2))
        ctx_pool = es.enter_context(tc.tile_pool(name="ctxp", bufs=5 if _pf2 else 4))
        ctxT_pool = es.enter_context(
            tc.tile_pool(name="ctxTp", bufs=3 if _pf2 else 2)
        )
        epool = es.enter_context(tc.tile_pool(name="epool", bufs=2))
        stg_pool = es.enter_context(tc.tile_pool(name="stg", bufs=2))
        small = es.enter_context(tc.tile_pool(name="small", bufs=8))
        bc_pool = es.enter_context(tc.tile_pool(name="bcast", bufs=3))
        dram = es.enter_context(tc.tile_pool(name="dram", bufs=2, space="DRAM"))
        ps_sim = es.enter_context(tc.tile_pool(name="ps_sim", bufs=1, space="PSUM"))
        ps_c2q = es.enter_context(tc.tile_pool(name="ps_c2q", bufs=2, space="PSUM"))
        ps_et = es.enter_context(tc.tile_pool(name="ps_et", bufs=1, space="PSUM"))
        ps_q2c = es.enter_context(tc.tile_pool(name="ps_q2c", bufs=1, space="PSUM"))

        _g4eng = nc.gpsimd if os.environ.get("KBENCH_G4ENG") == "gpsimd" else nc.vector

        _psbc = os.environ.get("KBENCH_PSBC") == "1"

        identity = singles.tile([128, 128], f16)
        nc.sync.dma_start(identity, ident_d[:, :])
        ones_col = singles.tile([128, 1], f32)
        nc.vector.memset(ones_col, 1.0)
        ones_row = singles.tile([1, 128], f16)
        nc.vector.memset(ones_row, 1.0)
        xq_t = singles.tile([Q, BPC], f32)
        nc.sync.dma_start(xq_t, xq_d[:, :])
        exc_t = singles.tile([128, BPC, CBLK], f32)
        nc.sync.dma_start(exc_t, exc_d[:, :, :])

        def issue_loads(b):
            # ring budget: gpsimd = all loads + bounce (~15MB), sync = g2+g3
            # out (12.6MB), scalar = g4 out (6.3MB, issued only when its deps
            # are long resolved so the ACT sequencer never parks).
            ctxT_t = ctxT_pool.tile([128, DBLK, C], f16, tag="ctxT")
            qwT_t = big.tile([128, DBLK, Q], f16, tag="qwT")
            if b == 0 and os.environ.get("KBENCH_B0SYNC") == "1":
                # batch 0 gates the whole pipeline: give its ctxT the empty
                # sync ring so both rings pull input concurrently at startup
                nc.sync.dma_start(ctxT_t, ctxT_d[b])
            else:
                # ctxT first: it gates the next batch's sim matmuls.
                nc.gpsimd.dma_start(ctxT_t, ctxT_d[b])
            nc.gpsimd.dma_start(qwT_t, qwT_d[b])
            qaug_t = big.tile([Q, QAUG], f16, tag="qaug")
            nc.gpsimd.dma_start(qaug_t, qaug_d[b])
            ctx_t = ctx_pool.tile([128, CBLK, D], f16, tag="ctx")
            nc.gpsimd.dma_start(ctx_t, ctx_d[b])
            return ctx_t, ctxT_t, qwT_t, qaug_t

        def q2c_mm(st):
            """PE q2c matmuls for an OLDER batch whose m2 weights resolved
            long ago -- the PE never waits here."""
            p_ctx, p_m2, p_msum, p_g4r = st
            if _psbc:
                q2c_full = ps_q2c.tile([128, QAUG], f32)
                q2c_ps = q2c_full[0:1, :]
            else:
                q2c_full = None
                q2c_ps = ps_q2c.tile([1, QAUG], f32)
            for blk in range(CBLK):
                for lo, hi in ((0, 512), (512, 768)):
                    nc.tensor.matmul(
                        q2c_ps[:, lo:hi],
                        lhsT=p_m2[:, blk : blk + 1],
                        rhs=p_ctx[:, blk, lo:hi],
                        start=(blk == 0),
                        stop=(blk == CBLK - 1),
                    )
            nc.tensor.matmul(
                q2c_ps[:, D : D + 1], lhsT=ones_col, rhs=p_msum, start=True, stop=True
            )
            return q2c_ps, q2c_full

        def q2c_norm(st, q2c_ps, q2c_full):
            p_ctx, p_m2, p_msum, p_g4r = st
            zr_t = small.tile([1, 1], f32, tag="zr")
            nc.vector.reciprocal(zr_t, q2c_ps[:, D : D + 1])
            q2c_sb = small.tile([1, D], f16, tag="q2c")
            nc.scalar.mul(q2c_sb, q2c_ps[:, 0:D], zr_t)
            if _psbc:
                # rank-1 PE broadcast into the same PSUM banks: ones^T @ row.
                # g4's DVE multiplies read the result straight from PSUM --
                # no DRAM bounce, no SBUF staging, no DMA latency.
                for lo, hi in ((0, 512), (512, D)):
                    nc.tensor.matmul(
                        q2c_full[:, lo:hi],
                        lhsT=ones_row,
                        rhs=q2c_sb[:, lo:hi],
                        start=True,
                        stop=True,
                    )
                return p_ctx, q2c_full[:, 0:D], p_g4r
            # broadcast q2c row to 128 partitions via DRAM bounce
            q2c_dram = dram.tile([1, D], f16)
            nc.gpsimd.dma_start(q2c_dram, q2c_sb)
            bcast_t = bc_pool.tile([128, D], f16, tag="bc")
            q2c_ap = q2c_dram[:, :]
            nc.gpsimd.dma_start(
                bcast_t,
                bass.AP(
                    tensor=q2c_ap.tensor, offset=q2c_ap.offset, ap=[[0, 128], [1, D]]
                ),
            )
            return p_ctx, bcast_t, p_g4r

        def q2c_tail(st):
            return q2c_norm(st, *q2c_mm(st))

        def g4_burst(st):
            """Multiplies only; the store is issued later (from ACT, once
            these mults are guaranteed done, so the issue never parks)."""
            pb_ctx, pb_bcast, pb_gr = st
            stg4 = stg_pool.tile([128, CBLK, D], f16, tag="stg4")
            for blk in range(CBLK):
                _g4eng.tensor_mul(stg4[:, blk, :], pb_ctx[:, blk, :], pb_bcast)
            return pb_gr, stg4

        pend_q2c = None    # batch b-1: m2 ready, q2c matmuls pending
        pend_g4 = None     # batch b-2: broadcast ready, g4 multiplies pending
        pend_store = None  # g4 staging tile whose store issue was deferred
        tiles = issue_loads(0)
        nxt = issue_loads(1) if _pf2 and BPC > 1 else None
        for b in range(BPC):
            ctx_t, ctxT_t, qwT_t, qaug_t = tiles
            # prefetch upcoming batches' inputs NOW, so their descriptors sit
            # ahead of this batch's late-resolving bounce DMAs in ring order
            if _pf2:
                if nxt is not None:
                    tiles = nxt
                nxt = issue_loads(b + 2) if b + 2 < BPC else None
            elif b + 1 < BPC:
                tiles = issue_loads(b + 1)
            g23_r = g23_d[b].rearrange("(p j) d -> p j d", j=CBLK)
            g4_r = g4_d[b].rearrange("(p j) d -> p j d", j=CBLK)

            # deferred g4 store: its multiplies finished last iteration, so
            # this ACT-ring issue finds its waits already satisfied
            if pend_store is not None:
                nc.scalar.dma_start(*pend_store)
                pend_store = None

            # ---- simT[q, c] = (query*wqc) @ ctx^T, then E = exp(simT + x_q)
            E_t = epool.tile([Q, C], f16, tag="E")
            for half in range(2):
                sim_ps = ps_sim.tile([Q, 512], f32, tag="sim")
                for k in range(DBLK):
                    nc.tensor.matmul(
                        sim_ps,
                        lhsT=qwT_t[:, k, :],
                        rhs=ctxT_t[:, k, half * 512 : (half + 1) * 512],
                        start=(k == 0),
                        stop=(k == DBLK - 1),
                    )
                nc.scalar.activation(
                    E_t[:, half * 512 : (half + 1) * 512],
                    sim_ps,
                    AF.Exp,
                    bias=xq_t[:, b : b + 1],
                    scale=1.0,
                )

            # ---- q2c stage for batch b-1, slotted right after sim so the
            # PE stream never idles waiting on this batch's max-chain
            pend_g4_new = q2c_tail(pend_q2c) if pend_q2c is not None else None

            # ---- maxE transposes + c2q matmuls + evac
            m_t = small.tile([128, CBLK], f32, tag="m")
            stg = stg_pool.tile([128, CBLK, 2 * D], f16, tag="stg")
            batch_et = os.environ.get("KBENCH_BATCHET") == "1"
            if batch_et:
                # all 8 transposes into one 1-bank PSUM tile, then a single
                # 3D reduce_max: 7 fewer DVE ops and far fewer sync edges,
                # and m2 is ready before the c2q loop even starts
                et_all = ps_et.tile([128, CBLK, 128], f16, tag="et")
                for blk in range(CBLK):
                    nc.tensor.transpose(
                        et_all[:, blk, :], E_t[:, blk * 128 : (blk + 1) * 128], identity
                    )
                nc.vector.reduce_max(m_t, et_all, axis=AX)
            for blk in range(CBLK):
                if not batch_et:
                    et_ps = ps_et.tile([128, 128], f16, tag="et")
                    nc.tensor.transpose(
                        et_ps, E_t[:, blk * 128 : (blk + 1) * 128], identity
                    )
                    nc.vector.reduce_max(m_t[:, blk : blk + 1], et_ps, axis=AX)

                c2q_ps = ps_c2q.tile([128, QAUG], f32)
                for lo, hi in ((0, 512), (512, QAUG)):
                    nc.tensor.matmul(
                        c2q_ps[:, lo:hi],
                        lhsT=E_t[:, blk * 128 : (blk + 1) * 128],
                        rhs=qaug_t[:, lo:hi],
                        start=True,
                        stop=True,
                    )

                rs_t = small.tile([128, 1], f32, tag="rs")
                nc.vector.reciprocal(rs_t, c2q_ps[:, D : D + 1])
                # g2 = c2q (normalized) -- ScalarE evac with fused scale
                nc.scalar.mul(stg[:, blk, 0:D], c2q_ps[:, 0:D], rs_t)

            # ---- q2c weights for THIS batch (consumed next iteration)
            m2_t = small.tile([128, CBLK], f16, tag="m2")
            nc.vector.tensor_mul(m2_t, m_t, exc_t[:, b, :])
            msum_t = small.tile([128, 1], f32, tag="msum")
            nc.vector.reduce_sum(msum_t, m2_t, axis=AX)

            last = b == BPC - 1
            if last and os.environ.get("KBENCH_TAIL") == "1":
                # ---- final batch: drain with maximum store/compute overlap
                st3 = (ctx_t, m2_t, msum_t, g4_r)
                q2c_ps3, q2c_full3 = q2c_mm(st3)
                half = CBLK // 2
                for blk in range(half):
                    nc.vector.tensor_mul(
                        stg[:, blk, D : 2 * D], stg[:, blk, 0:D], ctx_t[:, blk, :]
                    )
                # g4 of b-2 finished its mults last iteration's slot; store
                # it on sync ahead of the first g23 half
                s = g4_burst(pend_g4)
                nc.sync.dma_start(*s)
                nc.sync.dma_start(g23_r[:, 0:half, :], stg[:, 0:half, :])
                pend_g4_last = q2c_norm(st3, q2c_ps3, q2c_full3)
                for blk in range(half, CBLK):
                    nc.vector.tensor_mul(
                        stg[:, blk, D : 2 * D], stg[:, blk, 0:D], ctx_t[:, blk, :]
                    )
                # second g23 half drains on the scalar ring concurrently
                nc.scalar.dma_start(g23_r[:, half:, :], stg[:, half:, :])
                s = g4_burst(pend_g4_new)
                nc.scalar.dma_start(*s)
                s = g4_burst(pend_g4_last)
                nc.sync.dma_start(*s)
                continue

            if last and not _psbc:
                # start the final batch's q2c/broadcast before the g3 burst
                # so its g4 stage overlaps the remaining DVE work
                pend_g4_last = q2c_tail((ctx_t, m2_t, msum_t, g4_r))

            # g3 = ctx * g2, cheap f16 multiplies on DVE
            if os.environ.get("KBENCH_SPLITST") == "1":
                # ship [g2|g3] in two 1.5MB halves on different rings: the
                # first half starts draining 4 blocks earlier, and the store
                # traffic splits across Q10/Q1 instead of bursting on one
                half = CBLK // 2
                for blk in range(half):
                    nc.vector.tensor_mul(
                        stg[:, blk, D : 2 * D], stg[:, blk, 0:D], ctx_t[:, blk, :]
                    )
                nc.scalar.dma_start(g23_r[:, 0:half, :], stg[:, 0:half, :])
                for blk in range(half, CBLK):
                    nc.vector.tensor_mul(
                        stg[:, blk, D : 2 * D], stg[:, blk, 0:D], ctx_t[:, blk, :]
                    )
                nc.sync.dma_start(g23_r[:, half:, :], stg[:, half:, :])
            elif os.environ.get("KBENCH_FUSEG3") == "1":
                # all 8 blocks in one DVE instruction over 3D APs
                nc.vector.tensor_mul(
                    stg[:, :, D : 2 * D], stg[:, :, 0:D], ctx_t[:, :, :]
                )
                nc.sync.dma_start(g23_r, stg)
            else:
                # one contiguous 3MB store per batch, parked on idle Sync
                for blk in range(CBLK):
                    nc.vector.tensor_mul(
                        stg[:, blk, D : 2 * D], stg[:, blk, 0:D], ctx_t[:, blk, :]
                    )
                nc.sync.dma_start(g23_r, stg)

            if _psbc:
                # ---- 1-stage g4: batch b-1's broadcast lives in PSUM and
                # must be consumed before the next tail reuses those banks
                if pend_g4_new is not None:
                    pend_store = g4_burst(pend_g4_new)
                if last:
                    t3 = q2c_tail((ctx_t, m2_t, msum_t, g4_r))
                    s = g4_burst(t3)
                    nc.sync.dma_start(*s)
            else:
                # ---- g4 for batch b-2 (broadcast resolved a full stage ago)
                if pend_g4 is not None:
                    pend_store = g4_burst(pend_g4)
                pend_g4 = pend_g4_new
            pend_q2c = (ctx_t, m2_t, msum_t, g4_r)

        if _psbc:
            if pend_store is not None:
                nc.scalar.dma_start(*pend_store)
        elif os.environ.get("KBENCH_TAIL") != "1":
            # drain the pipeline: g4 of batches BPC-2 and BPC-1
            if pend_store is not None:
                nc.scalar.dma_start(*pend_store)
            s1 = g4_burst(pend_g4)
            nc.scalar.dma_start(*s1)
            s2 = g4_burst(pend_g4_last)
            nc.scalar.dma_start(*s2)

    if not sim:
        _split_waits(nc)
    return nc


def prepare_inputs(context, context_mask, query, query_mask, wq, wc, wqc):
    """Host-side prep: fold weights/masks, transpose, shard across 8 cores."""
    ctx = np.ascontiguousarray(np.asarray(context, dtype=np.float32))
    qry = np.ascontiguousarray(np.asarray(query, dtype=np.float32))
    cmask = np.asarray(context_mask)
    qmask = np.asarray(query_mask)
    wq = np.asarray(wq, dtype=np.float32)
    wc = np.asarray(wc, dtype=np.float32)
    wqc = np.asarray(wqc, dtype=np.float32)

    qw = qry * wqc[None, None, :]
    xq = np.einsum("bqd,d->bq", qry, wq).astype(np.float32)
    xc = np.einsum("bcd,d->bc", ctx, wc).astype(np.float32)
    # Mask folding: masked q -> -1e30 bias inside exp; masked c -> exc=0.
    xq_eff = np.where(qmask == 1, xq, np.float32(-1e30)).astype(np.float32)
    with np.errstate(over="ignore"):
        exc = np.exp(
            np.where(cmask == 1, xc, np.float32(-np.inf)), dtype=np.float32
        )

    # c-axis permutation: E-column e <-> context row rho(e) = 8*(e%128) + e//128.
    # Then the et-transpose output (partition p of chunk t <-> e = t*128+p)
    # lands exactly in the packed ctx layout (partition p, chunk j <-> row 8p+j).
    rho = (8 * (np.arange(C) % 128) + np.arange(C) // 128).astype(np.int64)
    # pctx[b, p, j, :] = ctx[b, 8p+j, :]  (contiguous 12KB per partition)
    pctx = np.ascontiguousarray(ctx.reshape(B, 128, CBLK, D).astype(np.float16))
    # pctxT[b, p, k, e] = ctx[b, rho(e), k*128+p]
    ctx_rho = ctx[:, rho, :]                          # [B, C(e-order), D]
    pctxT = np.ascontiguousarray(
        ctx_rho.transpose(0, 2, 1).reshape(B, DBLK, 128, C).transpose(0, 2, 1, 3)
    ).astype(np.float16)
    # pqwT[b, p, k, q] = qw[b, q, k*128+p]
    qwT = np.ascontiguousarray(qw.transpose(0, 2, 1).astype(np.float32))
    pqwT = np.ascontiguousarray(
        qwT.reshape(B, DBLK, 128, Q).transpose(0, 2, 1, 3)
    ).astype(np.float16)
    qaug = np.concatenate(
        [qry, np.ones((B, Q, 1), np.float32), np.zeros((B, Q, 1), np.float32)],
        axis=2,
    ).astype(np.float16)

    in_maps = []
    for i in range(N_CORES):
        sl = slice(i * BPC, (i + 1) * BPC)
        in_maps.append(
            {
                "ctx": pctx[sl],
                "ctxT": pctxT[sl],
                "qwT": pqwT[sl],
                "qaug": np.ascontiguousarray(qaug[sl]),
                "xq": np.ascontiguousarray(xq_eff[sl].T),
                "exc": np.ascontiguousarray(
                    exc[sl].reshape(BPC, 128, CBLK).transpose(1, 0, 2)
                ),
                "ident": np.eye(128, dtype=np.float16),
            }
        )
    return in_maps


def kernel(context, context_mask, query, query_mask, wq, wc, wqc):
    global LAST_RESULT
    from concourse.bass_utils import run_bass_kernel_spmd

    in_maps = prepare_inputs(
        context, context_mask, query, query_mask, wq, wc, wqc
    )
    nc = build_bass()
    res = run_bass_kernel_spmd(nc, in_maps, core_ids=list(range(N_CORES)))
    LAST_RESULT = res
    out = np.empty((B, C, 4 * D), dtype=np.float32)
    # g1 = context verbatim (never moved through the device)
    out[:, :, 0:D] = np.asarray(context, dtype=np.float32)
    for i in range(N_CORES):
        sl = slice(i * BPC, (i + 1) * BPC)
        out[sl, :, D : 3 * D] = (
            res.results[i]["g23"].reshape(BPC, C, 2 * D).astype(np.float32)
        )
        out[sl, :, 3 * D :] = (
            res.results[i]["g4"].reshape(BPC, C, D).astype(np.float32)
        )
    return out

